# revision 35
# baseline (speedup 1.0000x reference)
"""GAU attention (gated attention unit) Trainium2 Bass kernel.

Reference computation (B=2, S=2048, D=1024, H=16, DH=64):
    q = (hs @ Wq + bq), k = (hs @ Wk + bk), v = (hs @ Wv + bv)   per-head [B,S,H,DH]
    scores = q k^T / sqrt(DH);  probs = softmax(scores, axis=k)
    gating = sigmoid(gf * mean_d(hs) + gb)          # [B, S] per (batch, query)
    ctx = (probs * gating) @ v;  out = ctx @ Wo + bo

Sharding: 8 cores = 2 batches x 4 head-groups (4 heads each).  Each core
computes out^T partial [D, S] for its (batch, head-group); host sums the 4
partials per batch, applies the per-(batch,query) gating scalar (it commutes
to the output), transposes, and adds bo.

Per-core dataflow (all matmuls bf16 with fp32 PSUM accumulation):
  - hs^T [D,S] staged bf16 (host transposes + casts).
  - Q^T,K^T [256,S]: lhsT=W tiles (stationary), rhs=hs^T.  Layout: pair p of
    heads stacked on partitions (head A dh on 0:64, head B on 64:128).
  - K-proj/V-proj/Q-proj are emitted just-in-time inside the first q-chunk's
    attention loop so the exp (ACT) stream starts ~10us into the kernel.
  - scores^T [k,q] per (pr, kt): two row-packed (tile_position (0,0)/(64,0))
    K=64 matmuls -> exp on ACT (scale=1/8) -> E^T bf16.  Both head-pair
    groups (pr=0,1) are interleaved per kt so ACT stays saturated.
  - softmax denom: E^T ktiles folded into 2 partial sums per pr -- even kt
    on DVE, odd kt on GPSIMD (parallel engines) -- then one col-packed
    ones-matmul pair broadcasts both heads' denominators into one [128,GQ]
    PSUM tile (accumulating even+odd partials), one reciprocal, one multiply.
  - AV: col-packed (tile_position (0,0)/(0,64)) matmuls, V stationary,
    E^T streaming -> ctx^T accumulated over ktiles in a single PSUM bank per
    pr (disjoint partition ranges; skip_group_check).
  - O-proj lhsT=Wo, rhs=ctx^T (bf16, already 1/denom-scaled) -> out^T.
"""

import sys

for _p in ("/opt/trn_rl_repo", "/root/.axon_site/_ro/trn_rl_repo"):
    if _p not in sys.path:
        sys.path.append(_p)

from contextlib import ExitStack

import ml_dtypes
import numpy as np

import concourse.bass as bass
import concourse.mybir as mybir
import concourse.tile as tile
from concourse import bacc
from concourse.bass_utils import run_bass_kernel_spmd

BF16 = mybir.dt.bfloat16
F32 = mybir.dt.float32
AF = mybir.ActivationFunctionType
OP = mybir.AluOpType

B, S, D, H = 2, 2048, 1024, 16
DH = 64
LN2 = float(np.log(2.0))
LOG2E = float(np.log2(np.e))
HPC = 4  # heads per core
GD = HPC * DH  # 256 (head-group width)
NCORES = 8
NDT = D // 128  # 8 contraction tiles over D


def _build(ctx: ExitStack, tc: "tile.TileContext", io: dict, s: int):
    nc = tc.nc
    GQ = min(512, s)
    NQC = s // GQ  # q chunks
    NKT = s // 128  # k tiles

    hsT, wq, wk, wv, wo = io["hsT"], io["wq"], io["wk"], io["wv"], io["wo"]
    bq, bk, bv, outT = io["bq"], io["bk"], io["bv"], io["outT"]

    consts = ctx.enter_context(tc.tile_pool(name="consts", bufs=1))
    sb = ctx.enter_context(tc.tile_pool(name="sb", bufs=1))
    etp = ctx.enter_context(tc.tile_pool(name="etp", bufs=8))
    ksp = ctx.enter_context(tc.tile_pool(name="ksp", bufs=2))
    outp = ctx.enter_context(tc.tile_pool(name="outp", bufs=8))
    # PSUM budget: 2x2 (scores, 2-bank tiles) + 2 (ctx, one bank per pr via
    # disjoint-partition accumulation groups) + 2 (vproj/denom/o-proj) = 8
    ps_mm = ctx.enter_context(tc.tile_pool(name="ps_mm", bufs=2, space="PSUM"))
    ps_ctx = ctx.enter_context(tc.tile_pool(name="ps_ctx", bufs=2, space="PSUM"))
    ps_o = ctx.enter_context(tc.tile_pool(name="ps_o", bufs=2, space="PSUM"))

    # ---- constants ----
    ones128 = consts.tile([128, 128], BF16, tag="ones128", name="ones128")
    nc.vector.memset(ones128[:], 1.0)

    bq_sb = consts.tile([128, 2], F32, tag="bq", name="bq")
    nc.sync.dma_start(bq_sb[:], bq.rearrange("(m p) -> p m", p=128))
    bk_sb = consts.tile([128, 2], F32, tag="bk", name="bk")
    nc.sync.dma_start(bk_sb[:], bk.rearrange("(m p) -> p m", p=128))
    # explicit zero bias for Exp, written by DVE so the wait merges with the
    # DVE wait the exps already carry
    zbias = consts.tile([128, 1], F32, tag="zbias", name="zbias")
    nc.vector.memset(zbias[:], 0.0)
    # dummy exp as the very first ACT instruction: pulls the ~2.7us
    # ACT_TABLE_LOAD into the DMA-wait window
    warm = consts.tile([1, 1], F32, tag="warm", name="warm")
    nc.scalar.activation(warm[:], zbias[0:1, 0:1], AF.Exp, bias=zbias[0:1, 0:1], scale=1.0)

    # bv arrives pre-broadcast [128, GD] from the host
    bv_bc = consts.tile([128, GD], F32, tag="bvbc", name="bvbc")
    nc.sync.dma_start(bv_bc[:], bv[:, :])

    # ---- weights + hs^T staged: host pre-shuffles so every DMA is a large
    # fully-contiguous 2D block (4KB+ per partition line).  Weight tensors
    # live as [128, (d, GD)]; hs^T as [128, (chunk, d, GQ)]. ----
    wk_all = consts.tile([128, NDT * GD], BF16, tag="wk", name="wk")
    wq_all = consts.tile([128, NDT * GD], BF16, tag="wq", name="wq")
    wv_all = consts.tile([128, NDT * GD], BF16, tag="wv", name="wv")
    CW = NDT * GQ  # 4096 columns per hs chunk block
    hsT_all = sb.tile([128, NDT * s], BF16, tag="hsT", name="hsT")
    wk_sb = [wk_all[:, d * GD : (d + 1) * GD] for d in range(NDT)]
    wq_sb = [wq_all[:, d * GD : (d + 1) * GD] for d in range(NDT)]
    wv_sb = [wv_all[:, d * GD : (d + 1) * GD] for d in range(NDT)]

    def hsq(d, qc):  # [128, GQ] tile of hs^T for (d-tile, q-chunk)
        off = qc * CW + d * GQ
        return hsT_all[:, off : off + GQ]

    def hsv(d, kt):  # [128, 128] tile of hs^T for (d-tile, k-tile)
        c, r = divmod(kt, 4)
        off = c * CW + d * GQ + r * 128
        return hsT_all[:, off : off + 128]

    # critical-path pieces (wk + hs chunk0 + wq) split in halves across the
    # two HW DGE rings; the scalar (ACT) ring carries ONLY critical pieces so
    # the exp stream isn't queued behind DMA issues
    HW_ = 4 * GD  # half of a weight block
    nc.sync.dma_start(wk_all[:, 0:HW_], wk[:, 0:HW_])
    nc.sync.dma_start(hsT_all[:, 0 : CW // 2], hsT[:, 0 : CW // 2])
    nc.scalar.dma_start(wk_all[:, HW_ : 2 * HW_], wk[:, HW_ : 2 * HW_])
    nc.scalar.dma_start(hsT_all[:, CW // 2 : CW], hsT[:, CW // 2 : CW])
    nc.scalar.dma_start(wq_all[:], wq[:, :])
    nc.sync.dma_start(wv_all[:], wv[:, :])
    for c in range(1, NQC):
        nc.sync.dma_start(hsT_all[:, c * CW : (c + 1) * CW], hsT[:, c * CW : (c + 1) * CW])
    wo_sb = [consts.tile([128, D], BF16, tag=f"wo{p}", name=f"wo{p}") for p in range(2)]
    for p in range(2):
        nc.sync.dma_start(wo_sb[p][:], wo[p * 128 : (p + 1) * 128, :])

    qT_sb = [sb.tile([128, s], BF16, tag=f"qT{m}", name=f"qT{m}") for m in range(2)]
    kT_sb = [sb.tile([128, s], BF16, tag=f"kT{m}", name=f"kT{m}") for m in range(2)]
    v_sb = [sb.tile([128, GD], BF16, tag=f"v{st}", name=f"v{st}") for st in range(NKT)]

    def kproj(m, c):
        ms = slice(m * 128, (m + 1) * 128)
        cc = slice(c * GQ, (c + 1) * GQ)
        p = ps_o.tile([128, GQ], F32, tag="po", name=f"kp{m}{c}")
        for d in range(NDT):
            nc.tensor.matmul(
                p[:], lhsT=wk_sb[d][:, ms], rhs=hsq(d, c),
                start=(d == 0), stop=(d == NDT - 1),
            )
        nc.vector.tensor_scalar_add(kT_sb[m][:, cc], p[:], bk_sb[:, m : m + 1])

    def qproj(m, qc):
        ms = slice(m * 128, (m + 1) * 128)
        cc = slice(qc * GQ, (qc + 1) * GQ)
        p = ps_o.tile([128, GQ], F32, tag="po", name=f"qp{m}{qc}")
        for d in range(NDT):
            nc.tensor.matmul(
                p[:], lhsT=wq_sb[d][:, ms], rhs=hsq(d, qc),
                start=(d == 0), stop=(d == NDT - 1),
            )
        nc.vector.tensor_scalar_add(qT_sb[m][:, cc], p[:], bq_sb[:, m : m + 1])

    qchains = {}

    def qproj_half(m, qc, half):
        ms = slice(m * 128, (m + 1) * 128)
        if half == 0:
            qchains[(m, qc)] = ps_o.tile([128, GQ], F32, tag="po", name=f"qph{m}{qc}")
        p = qchains[(m, qc)]
        for d in range(half * 4, half * 4 + 4):
            nc.tensor.matmul(
                p[:], lhsT=wq_sb[d][:, ms], rhs=hsq(d, qc),
                start=(d == 0), stop=(d == NDT - 1),
            )
        if half == 1:
            cc = slice(qc * GQ, (qc + 1) * GQ)
            nc.vector.tensor_scalar_add(qT_sb[m][:, cc], p[:], bq_sb[:, m : m + 1])

    def vproj(kt):
        vp = ps_o.tile([128, GD], F32, tag="po", name=f"vp{kt}")
        for d in range(NDT):
            nc.tensor.matmul(
                vp[:], lhsT=hsv(d, kt), rhs=wv_sb[d][:],
                start=(d == 0), stop=(d == NDT - 1),
            )
        nc.vector.tensor_tensor(v_sb[kt][:], vp[:], bv_bc[:], op=OP.add)

    # ---- PE warm-up: ~3.5us of dependency-free matmuls so the HAM clock
    # gate opens before the real (DMA-gated) projections run ----
    warm_ps = ps_o.tile([128, 128], F32, tag="po", name="warm_ps")
    for i in range(24):
        nc.tensor.matmul(
            warm_ps[:], lhsT=ones128[:], rhs=ones128[:],
            start=(i == 0), stop=(i == 23),
        )

    # ---- prologue: just the pr0 projections; pr1's come as slot-0 filler
    # so the first exp fires after only two projection chains ----
    kproj(0, 0)
    qproj(0, 0)

    def oproj_unit(mt, ctx_sc, cs, copy_eng, dma_eng):
        ms = slice(mt * 128, (mt + 1) * 128)
        o_ps = ps_o.tile([128, GQ], F32, tag="po", name="po")
        for pr in range(2):
            nc.tensor.matmul(
                o_ps[:], lhsT=wo_sb[pr][:, ms], rhs=ctx_sc[pr][:],
                start=(pr == 0), stop=(pr == 1),
            )
        ost = outp.tile([128, GQ], BF16, tag="ost", name="ost")
        copy_eng(ost[:], o_ps[:])
        dma_eng.dma_start(outT[ms, cs], ost[:])

    # ---- per q-chunk attention, both head-pair groups interleaved per kt;
    # O-proj of chunk qc-1 trickles into qc's slots as PE filler ----
    oproj_pending: list = []
    for qc in range(NQC):
        cs = slice(qc * GQ, (qc + 1) * GQ)
        ctx_ps = [ps_ctx.tile([128, GQ], F32, tag="ctx", name=f"ctx{pr}") for pr in range(2)]
        # per (pr, kt-parity) partial exp-sums, folded on DVE
        ks = [[None, None], [None, None]]
        ets = [[None] * NKT, [None] * NKT]
        for kt in range(NKT + 1):
            sps = [None, None]
            for pr in range(2):
                if kt < NKT:
                    ks_ = slice(kt * 128, (kt + 1) * 128)
                    sp = ps_mm.tile([128, 2 * GQ], F32, tag="smm", name="smm")
                    sps[pr] = sp
                    nc.tensor.matmul(
                        sp[:, 0:GQ], lhsT=kT_sb[pr][0:64, ks_], rhs=qT_sb[pr][0:64, cs],
                        tile_position=(0, 0), start=True, stop=True,
                    )
                    nc.tensor.matmul(
                        sp[:, GQ : 2 * GQ], lhsT=kT_sb[pr][64:128, ks_], rhs=qT_sb[pr][64:128, cs],
                        tile_position=(64, 0), start=True, stop=True,
                    )
                if pr == 0 and qc == 0 and kt == 0:
                    # pr1's projections must precede pr1's first scores in
                    # program order (tile deps are program-order-based)
                    kproj(1, 0)
                    qproj(1, 0)
                    vproj(0)
                if kt > 0:
                    pv = kt - 1
                    et = ets[pr][pv]
                    nc.tensor.matmul(
                        ctx_ps[pr][0:64, :], lhsT=v_sb[pv][:, pr * 128 : pr * 128 + 64],
                        rhs=et[:, 0:GQ], tile_position=(0, 0),
                        start=(pv == 0), stop=(pv == NKT - 1),
                        skip_group_check=True,
                    )
                    nc.tensor.matmul(
                        ctx_ps[pr][64:128, :], lhsT=v_sb[pv][:, pr * 128 + 64 : pr * 128 + 128],
                        rhs=et[:, GQ : 2 * GQ], tile_position=(0, 64),
                        start=(pv == 0), stop=(pv == NKT - 1),
                        skip_group_check=True,
                    )
            # just-in-time projection filler after the slot's critical MMs:
            # its latency hides under the two exps of this slot
            if kt < NKT:
                if qc == 0:
                    if kt + 1 < NKT:
                        vproj(kt + 1)
                    if kt in (1, 5, 9):
                        kproj(0, kt // 4 + 1)
                    elif kt in (2, 6, 10):
                        kproj(1, kt // 4 + 1)
                    elif kt == 12:
                        qproj(0, 1)
                    elif kt == 13:
                        qproj(1, 1)
                else:
                    if qc < NQC - 1:
                        if kt in (5, 6):
                            qproj_half(0, qc + 1, kt - 5)
                        elif kt in (7, 8):
                            qproj_half(1, qc + 1, kt - 7)
                    if oproj_pending and (kt < 5 or kt > 8):
                        oproj_pending.pop(0)()
            for pr in range(2):
                if kt < NKT:
                    et = etp.tile([128, 2 * GQ], BF16, tag="et", name="et")
                    ets[pr][kt] = et
                    # scores arrive in log2 space (log2e/8 folded into Wq):
                    # exp(ln2*y) = 2^y on ACT
                    nc.scalar.activation(et[:], sps[pr][:], AF.Exp, bias=zbias[:, 0:1], scale=LN2)
                    par = kt % 2
                    if kt >= 2:
                        if kt < 4:
                            # first fold is out-of-place (no seed copy needed)
                            kst = ksp.tile([128, 2 * GQ], BF16, tag=f"ks{pr}{par}", name=f"ks{pr}{par}")
                            ks[pr][par] = kst
                            nc.vector.tensor_tensor(kst[:], ets[pr][par][:], et[:], op=OP.add)
                        else:
                            kst = ks[pr][par]
                            nc.vector.tensor_tensor(kst[:], kst[:], et[:], op=OP.add)

        # softmax denominators: col-packed ones-matmul pair broadcasts both
        # heads into one bank, accumulating even+odd partial sums
        def denom_ctx(pr):
            db = ps_o.tile([128, GQ], F32, tag="po", name=f"db{pr}")
            for par in range(2):
                for hh in range(2):
                    nc.tensor.matmul(
                        db[hh * 64 : (hh + 1) * 64, :],
                        lhsT=ones128[:, hh * 64 : (hh + 1) * 64],
                        rhs=ks[pr][par][:, hh * GQ : (hh + 1) * GQ],
                        tile_position=(0, hh * 64),
                        start=(par == 0), stop=(par == 1),
                        skip_group_check=True,
                    )
            r = ksp.tile([128, GQ], F32, tag=f"r{pr}", name=f"r{pr}")
            nc.vector.reciprocal_approx_fast(r[:], db[:])
            sc = sb.tile([128, GQ], BF16, tag=f"ctxs{pr}_{qc % 2}", name=f"ctxs{pr}_{qc % 2}")
            nc.vector.tensor_tensor(sc[:], ctx_ps[pr][:], r[:], op=OP.mult)
            return sc

        if qc < NQC - 1:
            # deferred into the next chunk's slots as PE filler
            ctx_sc = [denom_ctx(0), denom_ctx(1)]
            oproj_pending = [
                (lambda mt=mt, sc2=list(ctx_sc), c=cs: oproj_unit(
                    mt, sc2, c, nc.vector.tensor_copy, nc.sync))
                for mt in range(D // 128)
            ]
        else:
            # tail: pr0's half of the O-projection overlaps pr1's softmax
            # drain; pr1's half is added on DVE, outputs on both DMA rings
            sc0 = denom_ctx(0)
            osts = []
            for mt in range(D // 128):
                o_ps = ps_o.tile([128, GQ], F32, tag="po", name="po")
                nc.tensor.matmul(o_ps[:], lhsT=wo_sb[0][:, mt * 128 : (mt + 1) * 128],
                                 rhs=sc0[:], start=True, stop=True)
                ost = outp.tile([128, GQ], BF16, tag="ost", name="ost")
                copy_eng = nc.vector.tensor_copy if mt % 2 == 0 else nc.scalar.copy
                copy_eng(ost[:], o_ps[:])
                osts.append(ost)
            sc1 = denom_ctx(1)
            for mt in range(D // 128):
                ms = slice(mt * 128, (mt + 1) * 128)
                o_ps = ps_o.tile([128, GQ], F32, tag="po", name="po")
                nc.tensor.matmul(o_ps[:], lhsT=wo_sb[1][:, ms], rhs=sc1[:],
                                 start=True, stop=True)
                nc.vector.tensor_tensor(osts[mt][:], osts[mt][:], o_ps[:], op=OP.add)
                dma_eng = nc.sync if mt % 2 == 0 else nc.scalar
                dma_eng.dma_start(outT[ms, cs], osts[mt][:])


def build_gau_nc(s: int = S, debug: bool = False):
    nc = bacc.Bacc("TRN2", target_bir_lowering=False, debug=debug, num_devices=NCORES)
    io = {
        "hsT": nc.dram_tensor("hsT", [128, (D // 128) * s], BF16, kind="ExternalInput").ap(),
        "wq": nc.dram_tensor("wq", [128, (D // 128) * GD], BF16, kind="ExternalInput").ap(),
        "wk": nc.dram_tensor("wk", [128, (D // 128) * GD], BF16, kind="ExternalInput").ap(),
        "wv": nc.dram_tensor("wv", [128, (D // 128) * GD], BF16, kind="ExternalInput").ap(),
        "wo": nc.dram_tensor("wo", [GD, D], BF16, kind="ExternalInput").ap(),
        "bq": nc.dram_tensor("bq", [GD], F32, kind="ExternalInput").ap(),
        "bk": nc.dram_tensor("bk", [GD], F32, kind="ExternalInput").ap(),
        "bv": nc.dram_tensor("bv", [128, GD], F32, kind="ExternalInput").ap(),
        "outT": nc.dram_tensor("outT", [D, s], BF16, kind="ExternalOutput").ap(),
    }
    with tile.TileContext(nc) as tc:
        with ExitStack() as ctx:
            _build(ctx, tc, io, s)
    nc.compile()
    return nc


def make_in_maps(hidden_states, Wq, bq, Wk, bk, Wv, bv, Wo, gating_factor, gating_bias):
    """Shard full inputs into 8 per-core input maps (host-side prep)."""
    bf = ml_dtypes.bfloat16
    f32 = np.float32
    hs = np.asarray(hidden_states, f32)
    Wq, Wk, Wv, Wo = (np.asarray(a, f32) for a in (Wq, Wk, Wv, Wo))
    bq, bk, bv = (np.asarray(a, f32) for a in (bq, bk, bv))

    # hs^T pre-shuffled to [128, (chunk, d, 512)] so the device DMA is a
    # plain contiguous 2D block; weights to [128, (d, GD)] likewise
    def shuf_hs(a):  # a: [S, D]
        return np.ascontiguousarray(
            a.reshape(S // 512, 512, D // 128, 128).transpose(3, 0, 2, 1).reshape(128, -1)
        ).astype(bf)

    def shuf_w(w):  # w: [D, GD]
        return np.ascontiguousarray(
            w.reshape(D // 128, 128, GD).transpose(1, 0, 2).reshape(128, -1)
        ).astype(bf)

    hsT_b = [shuf_hs(hs[b]) for b in range(B)]
    in_maps = []
    for c in range(NCORES):
        b, g = divmod(c, NCORES // B)
        cols = slice(g * GD, (g + 1) * GD)
        in_maps.append(
            {
                "hsT": hsT_b[b],
                "wq": shuf_w(Wq[:, cols] * np.float32(LOG2E / 8.0)),
                "wk": shuf_w(Wk[:, cols]),
                "wv": shuf_w(Wv[:, cols]),
                "wo": np.ascontiguousarray(Wo[cols, :]).astype(bf),
                "bq": np.ascontiguousarray(bq[cols] * np.float32(LOG2E / 8.0)),
                "bk": np.ascontiguousarray(bk[cols]),
                "bv": np.ascontiguousarray(np.broadcast_to(bv[cols], (128, GD))),
            }
        )
    return in_maps


_NC_CACHE: dict = {}


def _get_nc(s: int = S):
    if s not in _NC_CACHE:
        _NC_CACHE[s] = build_gau_nc(s)
    return _NC_CACHE[s]


def run_gau(in_maps, **kwargs):
    nc = _get_nc(S)
    return run_bass_kernel_spmd(nc, in_maps, core_ids=list(range(NCORES)), **kwargs)


def assemble_output(results, bo, gating):
    """Sum per-batch head-group partials, apply gating, transpose, add bo."""
    bo = np.asarray(bo, np.float32)
    gpb = NCORES // B
    out = np.empty((B, S, D), np.float32)
    for b in range(B):
        acc = results[gpb * b]["outT"].astype(np.float32)
        for g in range(1, gpb):
            acc = acc + results[gpb * b + g]["outT"].astype(np.float32)
        out[b] = acc.T * gating[b][:, None] + bo[None, :]
    return out


def kernel(hidden_states, Wq, bq, Wk, bk, Wv, bv, Wo, bo, gating_factor, gating_bias):
    in_maps = make_in_maps(
        hidden_states, Wq, bq, Wk, bk, Wv, bv, Wo, gating_factor, gating_bias
    )
    hs = np.asarray(hidden_states, np.float32)
    gf = np.float32(np.asarray(gating_factor, np.float32)[0])
    gb = np.float32(np.asarray(gating_bias, np.float32)[0])
    gating = 1.0 / (1.0 + np.exp(-(gf * hs.mean(axis=-1) + gb)))  # [B, S]
    res = run_gau(in_maps)
    return assemble_output(res.results, bo, gating)


# revision 36
# speedup vs baseline: 1.0218x; 1.0218x over previous
"""GAU attention (gated attention unit) Trainium2 Bass kernel.

Reference computation (B=2, S=2048, D=1024, H=16, DH=64):
    q = (hs @ Wq + bq), k = (hs @ Wk + bk), v = (hs @ Wv + bv)   per-head [B,S,H,DH]
    scores = q k^T / sqrt(DH);  probs = softmax(scores, axis=k)
    gating = sigmoid(gf * mean_d(hs) + gb)          # [B, S] per (batch, query)
    ctx = (probs * gating) @ v;  out = ctx @ Wo + bo

Sharding: 8 cores = 2 batches x 4 head-groups (4 heads each).  Each core
computes out^T partial [D, S] for its (batch, head-group); host sums the 4
partials per batch, applies the per-(batch,query) gating scalar (it commutes
to the output), transposes, and adds bo.

Per-core dataflow (all matmuls bf16 with fp32 PSUM accumulation):
  - hs^T [D,S] staged bf16 (host transposes + casts).
  - Q^T,K^T [256,S]: lhsT=W tiles (stationary), rhs=hs^T.  Layout: pair p of
    heads stacked on partitions (head A dh on 0:64, head B on 64:128).
  - K-proj/V-proj/Q-proj are emitted just-in-time inside the first q-chunk's
    attention loop so the exp (ACT) stream starts ~10us into the kernel.
  - scores^T [k,q] per (pr, kt): two row-packed (tile_position (0,0)/(64,0))
    K=64 matmuls -> exp on ACT (scale=1/8) -> E^T bf16.  Both head-pair
    groups (pr=0,1) are interleaved per kt so ACT stays saturated.
  - softmax denom: E^T ktiles folded into 2 partial sums per pr -- even kt
    on DVE, odd kt on GPSIMD (parallel engines) -- then one col-packed
    ones-matmul pair broadcasts both heads' denominators into one [128,GQ]
    PSUM tile (accumulating even+odd partials), one reciprocal, one multiply.
  - AV: col-packed (tile_position (0,0)/(0,64)) matmuls, V stationary,
    E^T streaming -> ctx^T accumulated over ktiles in a single PSUM bank per
    pr (disjoint partition ranges; skip_group_check).
  - O-proj lhsT=Wo, rhs=ctx^T (bf16, already 1/denom-scaled) -> out^T.
"""

import sys

for _p in ("/opt/trn_rl_repo", "/root/.axon_site/_ro/trn_rl_repo"):
    if _p not in sys.path:
        sys.path.append(_p)

from contextlib import ExitStack

import ml_dtypes
import numpy as np

import concourse.bass as bass
import concourse.mybir as mybir
import concourse.tile as tile
from concourse import bacc
from concourse.bass_utils import run_bass_kernel_spmd

BF16 = mybir.dt.bfloat16
F32 = mybir.dt.float32
AF = mybir.ActivationFunctionType
OP = mybir.AluOpType

B, S, D, H = 2, 2048, 1024, 16
DH = 64
LN2 = float(np.log(2.0))
LOG2E = float(np.log2(np.e))
HPC = 4  # heads per core
GD = HPC * DH  # 256 (head-group width)
NCORES = 8
NDT = D // 128  # 8 contraction tiles over D


def _build(ctx: ExitStack, tc: "tile.TileContext", io: dict, s: int):
    nc = tc.nc
    GQ = min(512, s)
    NQC = s // GQ  # q chunks
    NKT = s // 128  # k tiles

    hsT, wq, wk, wv, wo = io["hsT"], io["wq"], io["wk"], io["wv"], io["wo"]
    bq, bk, bv, outT = io["bq"], io["bk"], io["bv"], io["outT"]

    consts = ctx.enter_context(tc.tile_pool(name="consts", bufs=1))
    sb = ctx.enter_context(tc.tile_pool(name="sb", bufs=1))
    etp = ctx.enter_context(tc.tile_pool(name="etp", bufs=8))
    ksp = ctx.enter_context(tc.tile_pool(name="ksp", bufs=2))
    outp = ctx.enter_context(tc.tile_pool(name="outp", bufs=8))
    # PSUM budget: 2x2 (scores, 2-bank tiles) + 2 (ctx, one bank per pr via
    # disjoint-partition accumulation groups) + 2 (vproj/denom/o-proj) = 8
    ps_mm = ctx.enter_context(tc.tile_pool(name="ps_mm", bufs=2, space="PSUM"))
    ps_ctx = ctx.enter_context(tc.tile_pool(name="ps_ctx", bufs=2, space="PSUM"))
    ps_o = ctx.enter_context(tc.tile_pool(name="ps_o", bufs=2, space="PSUM"))

    # ---- constants ----
    ones128 = consts.tile([128, 128], BF16, tag="ones128", name="ones128")
    nc.vector.memset(ones128[:], 1.0)

    bq_sb = consts.tile([128, 2], F32, tag="bq", name="bq")
    nc.sync.dma_start(bq_sb[:], bq.rearrange("(m p) -> p m", p=128))
    bk_sb = consts.tile([128, 2], F32, tag="bk", name="bk")
    nc.sync.dma_start(bk_sb[:], bk.rearrange("(m p) -> p m", p=128))
    # explicit zero bias for Exp, written by DVE so the wait merges with the
    # DVE wait the exps already carry
    zbias = consts.tile([128, 1], F32, tag="zbias", name="zbias")
    nc.vector.memset(zbias[:], 0.0)
    # dummy exp as the very first ACT instruction: pulls the ~2.7us
    # ACT_TABLE_LOAD into the DMA-wait window
    warm = consts.tile([1, 1], F32, tag="warm", name="warm")
    nc.scalar.activation(warm[:], zbias[0:1, 0:1], AF.Exp, bias=zbias[0:1, 0:1], scale=1.0)

    # bv arrives pre-broadcast [128, GD] from the host
    bv_bc = consts.tile([128, GD], F32, tag="bvbc", name="bvbc")
    nc.sync.dma_start(bv_bc[:], bv[:, :])

    # ---- weights + hs^T staged: host pre-shuffles so every DMA is a large
    # fully-contiguous 2D block (4KB+ per partition line).  Weight tensors
    # live as [128, (d, GD)]; hs^T as [128, (chunk, d, GQ)]. ----
    wk_all = consts.tile([128, NDT * GD], BF16, tag="wk", name="wk")
    wq_all = consts.tile([128, NDT * GD], BF16, tag="wq", name="wq")
    wv_all = consts.tile([128, NDT * GD], BF16, tag="wv", name="wv")
    CW = NDT * GQ  # 4096 columns per hs chunk block
    hsT_all = sb.tile([128, NDT * s], BF16, tag="hsT", name="hsT")
    wk_sb = [wk_all[:, d * GD : (d + 1) * GD] for d in range(NDT)]
    wq_sb = [wq_all[:, d * GD : (d + 1) * GD] for d in range(NDT)]
    wv_sb = [wv_all[:, d * GD : (d + 1) * GD] for d in range(NDT)]

    def hsq(d, qc):  # [128, GQ] tile of hs^T for (d-tile, q-chunk)
        off = qc * CW + d * GQ
        return hsT_all[:, off : off + GQ]

    def hsv(d, kt):  # [128, 128] tile of hs^T for (d-tile, k-tile)
        c, r = divmod(kt, 4)
        off = c * CW + d * GQ + r * 128
        return hsT_all[:, off : off + 128]

    # critical-path pieces (wk + hs chunk0 + wq) split in halves across the
    # two HW DGE rings; the scalar (ACT) ring carries ONLY critical pieces so
    # the exp stream isn't queued behind DMA issues
    HW_ = 4 * GD  # half of a weight block
    nc.sync.dma_start(wk_all[:, 0:HW_], wk[:, 0:HW_])
    nc.sync.dma_start(hsT_all[:, 0 : CW // 2], hsT[:, 0 : CW // 2])
    nc.scalar.dma_start(wk_all[:, HW_ : 2 * HW_], wk[:, HW_ : 2 * HW_])
    nc.scalar.dma_start(hsT_all[:, CW // 2 : CW], hsT[:, CW // 2 : CW])
    nc.scalar.dma_start(wq_all[:], wq[:, :])
    nc.sync.dma_start(wv_all[:], wv[:, :])
    for c in range(1, NQC):
        nc.sync.dma_start(hsT_all[:, c * CW : (c + 1) * CW], hsT[:, c * CW : (c + 1) * CW])
    wo_sb = [consts.tile([128, D], BF16, tag=f"wo{p}", name=f"wo{p}") for p in range(2)]
    for p in range(2):
        nc.sync.dma_start(wo_sb[p][:], wo[p * 128 : (p + 1) * 128, :])

    qT_sb = [sb.tile([128, s], BF16, tag=f"qT{m}", name=f"qT{m}") for m in range(2)]
    kT_sb = [sb.tile([128, s], BF16, tag=f"kT{m}", name=f"kT{m}") for m in range(2)]
    v_sb = [sb.tile([128, GD], BF16, tag=f"v{st}", name=f"v{st}") for st in range(NKT)]

    def kproj(m, c):
        ms = slice(m * 128, (m + 1) * 128)
        cc = slice(c * GQ, (c + 1) * GQ)
        p = ps_o.tile([128, GQ], F32, tag="po", name=f"kp{m}{c}")
        for d in range(NDT):
            nc.tensor.matmul(
                p[:], lhsT=wk_sb[d][:, ms], rhs=hsq(d, c),
                start=(d == 0), stop=(d == NDT - 1),
            )
        nc.vector.tensor_scalar_add(kT_sb[m][:, cc], p[:], bk_sb[:, m : m + 1])

    def qproj(m, qc):
        ms = slice(m * 128, (m + 1) * 128)
        cc = slice(qc * GQ, (qc + 1) * GQ)
        p = ps_o.tile([128, GQ], F32, tag="po", name=f"qp{m}{qc}")
        for d in range(NDT):
            nc.tensor.matmul(
                p[:], lhsT=wq_sb[d][:, ms], rhs=hsq(d, qc),
                start=(d == 0), stop=(d == NDT - 1),
            )
        nc.vector.tensor_scalar_add(qT_sb[m][:, cc], p[:], bq_sb[:, m : m + 1])

    qchains = {}

    def qproj_half(m, qc, half):
        ms = slice(m * 128, (m + 1) * 128)
        if half == 0:
            qchains[(m, qc)] = ps_o.tile([128, GQ], F32, tag="po", name=f"qph{m}{qc}")
        p = qchains[(m, qc)]
        for d in range(half * 4, half * 4 + 4):
            nc.tensor.matmul(
                p[:], lhsT=wq_sb[d][:, ms], rhs=hsq(d, qc),
                start=(d == 0), stop=(d == NDT - 1),
            )
        if half == 1:
            cc = slice(qc * GQ, (qc + 1) * GQ)
            nc.vector.tensor_scalar_add(qT_sb[m][:, cc], p[:], bq_sb[:, m : m + 1])

    def vproj(kt):
        vp = ps_o.tile([128, GD], F32, tag="po", name=f"vp{kt}")
        for d in range(NDT):
            nc.tensor.matmul(
                vp[:], lhsT=hsv(d, kt), rhs=wv_sb[d][:],
                start=(d == 0), stop=(d == NDT - 1),
            )
        nc.vector.tensor_tensor(v_sb[kt][:], vp[:], bv_bc[:], op=OP.add)

    # ---- PE warm-up: ~3.5us of dependency-free matmuls so the HAM clock
    # gate opens before the real (DMA-gated) projections run ----
    warm_ps = ps_o.tile([128, 128], F32, tag="po", name="warm_ps")
    for i in range(24):
        nc.tensor.matmul(
            warm_ps[:], lhsT=ones128[:], rhs=ones128[:],
            start=(i == 0), stop=(i == 23),
        )

    # ---- prologue: just the pr0 projections; pr1's come as slot-0 filler
    # so the first exp fires after only two projection chains ----
    kproj(0, 0)
    qproj(0, 0)

    def oproj_unit(mt, ctx_sc, cs, copy_eng, dma_eng):
        ms = slice(mt * 128, (mt + 1) * 128)
        o_ps = ps_o.tile([128, GQ], F32, tag="po", name="po")
        for pr in range(2):
            nc.tensor.matmul(
                o_ps[:], lhsT=wo_sb[pr][:, ms], rhs=ctx_sc[pr][:],
                start=(pr == 0), stop=(pr == 1),
            )
        ost = outp.tile([128, GQ], BF16, tag="ost", name="ost")
        copy_eng(ost[:], o_ps[:])
        dma_eng.dma_start(outT[ms, cs], ost[:])

    # ---- per q-chunk attention, both head-pair groups interleaved per kt;
    # O-proj of chunk qc-1 trickles into qc's slots as PE filler ----
    oproj_pending: list = []
    for qc in range(NQC):
        cs = slice(qc * GQ, (qc + 1) * GQ)
        ctx_ps = [ps_ctx.tile([128, GQ], F32, tag="ctx", name=f"ctx{pr}") for pr in range(2)]
        # per (pr, kt-parity) partial exp-sums, folded on DVE
        ks = [[None, None], [None, None]]
        ets = [[None] * NKT, [None] * NKT]
        for kt in range(NKT + 1):
            sps = [None, None]
            for pr in range(2):
                if kt < NKT:
                    ks_ = slice(kt * 128, (kt + 1) * 128)
                    sp = ps_mm.tile([128, 2 * GQ], F32, tag="smm", name="smm")
                    sps[pr] = sp
                    nc.tensor.matmul(
                        sp[:, 0:GQ], lhsT=kT_sb[pr][0:64, ks_], rhs=qT_sb[pr][0:64, cs],
                        tile_position=(0, 0), start=True, stop=True,
                    )
                    nc.tensor.matmul(
                        sp[:, GQ : 2 * GQ], lhsT=kT_sb[pr][64:128, ks_], rhs=qT_sb[pr][64:128, cs],
                        tile_position=(64, 0), start=True, stop=True,
                    )
                if pr == 0 and kt < NKT:
                    # just-in-time projections keep PE fed while ACT drains exps
                    if qc == 0:
                        if kt == 0:
                            kproj(1, 0)
                            qproj(1, 0)
                            vproj(0)
                        if kt + 1 < NKT:
                            vproj(kt + 1)
                        if kt in (1, 5, 9):
                            kproj(0, kt // 4 + 1)
                        elif kt in (2, 6, 10):
                            kproj(1, kt // 4 + 1)
                        elif kt == 12:
                            qproj(0, 1)
                        elif kt == 13:
                            qproj(1, 1)
                    else:
                        if qc < NQC - 1:
                            if kt in (5, 6):
                                qproj_half(0, qc + 1, kt - 5)
                            elif kt in (7, 8):
                                qproj_half(1, qc + 1, kt - 7)
                        if oproj_pending and kt >= 1 and (kt < 5 or kt > 8):
                            oproj_pending.pop(0)()
                if kt > 0:
                    pv = kt - 1
                    et = ets[pr][pv]
                    nc.tensor.matmul(
                        ctx_ps[pr][0:64, :], lhsT=v_sb[pv][:, pr * 128 : pr * 128 + 64],
                        rhs=et[:, 0:GQ], tile_position=(0, 0),
                        start=(pv == 0), stop=(pv == NKT - 1),
                        skip_group_check=True,
                    )
                    nc.tensor.matmul(
                        ctx_ps[pr][64:128, :], lhsT=v_sb[pv][:, pr * 128 + 64 : pr * 128 + 128],
                        rhs=et[:, GQ : 2 * GQ], tile_position=(0, 64),
                        start=(pv == 0), stop=(pv == NKT - 1),
                        skip_group_check=True,
                    )
            for pr in range(2):
                if kt < NKT:
                    et = etp.tile([128, 2 * GQ], BF16, tag="et", name="et")
                    ets[pr][kt] = et
                    # scores arrive in log2 space (log2e/8 folded into Wq):
                    # exp(ln2*y) = 2^y on ACT
                    nc.scalar.activation(et[:], sps[pr][:], AF.Exp, bias=zbias[:, 0:1], scale=LN2)
                    par = kt % 2
                    if kt >= 2:
                        if kt < 4:
                            # first fold is out-of-place (no seed copy needed)
                            kst = ksp.tile([128, 2 * GQ], BF16, tag=f"ks{pr}{par}", name=f"ks{pr}{par}")
                            ks[pr][par] = kst
                            nc.vector.tensor_tensor(kst[:], ets[pr][par][:], et[:], op=OP.add)
                        else:
                            kst = ks[pr][par]
                            nc.vector.tensor_tensor(kst[:], kst[:], et[:], op=OP.add)

        # softmax denominators: col-packed ones-matmul pair broadcasts both
        # heads into one bank, accumulating even+odd partial sums
        def denom_ctx(pr):
            db = ps_o.tile([128, GQ], F32, tag="po", name=f"db{pr}")
            for par in range(2):
                for hh in range(2):
                    nc.tensor.matmul(
                        db[hh * 64 : (hh + 1) * 64, :],
                        lhsT=ones128[:, hh * 64 : (hh + 1) * 64],
                        rhs=ks[pr][par][:, hh * GQ : (hh + 1) * GQ],
                        tile_position=(0, hh * 64),
                        start=(par == 0), stop=(par == 1),
                        skip_group_check=True,
                    )
            r = ksp.tile([128, GQ], F32, tag=f"r{pr}", name=f"r{pr}")
            nc.vector.reciprocal_approx_fast(r[:], db[:])
            sc = sb.tile([128, GQ], BF16, tag=f"ctxs{pr}_{qc % 2}", name=f"ctxs{pr}_{qc % 2}")
            nc.vector.tensor_tensor(sc[:], ctx_ps[pr][:], r[:], op=OP.mult)
            return sc

        if qc < NQC - 1:
            # deferred into the next chunk's slots as PE filler
            ctx_sc = [denom_ctx(0), denom_ctx(1)]
            oproj_pending = [
                (lambda mt=mt, sc2=list(ctx_sc), c=cs: oproj_unit(
                    mt, sc2, c, nc.vector.tensor_copy, nc.sync))
                for mt in range(D // 128)
            ]
        else:
            # tail: pr0's half of the O-projection overlaps pr1's softmax
            # drain; pr1's half is added on DVE, outputs on both DMA rings
            sc0 = denom_ctx(0)
            osts = []
            for mt in range(D // 128):
                o_ps = ps_o.tile([128, GQ], F32, tag="po", name="po")
                nc.tensor.matmul(o_ps[:], lhsT=wo_sb[0][:, mt * 128 : (mt + 1) * 128],
                                 rhs=sc0[:], start=True, stop=True)
                ost = outp.tile([128, GQ], BF16, tag="ost", name="ost")
                copy_eng = nc.vector.tensor_copy if mt % 2 == 0 else nc.scalar.copy
                copy_eng(ost[:], o_ps[:])
                osts.append(ost)
            sc1 = denom_ctx(1)
            for mt in range(D // 128):
                ms = slice(mt * 128, (mt + 1) * 128)
                o_ps = ps_o.tile([128, GQ], F32, tag="po", name="po")
                nc.tensor.matmul(o_ps[:], lhsT=wo_sb[1][:, ms], rhs=sc1[:],
                                 start=True, stop=True)
                nc.vector.tensor_tensor(osts[mt][:], osts[mt][:], o_ps[:], op=OP.add)
                dma_eng = nc.sync if mt % 2 == 0 else nc.scalar
                dma_eng.dma_start(outT[ms, cs], osts[mt][:])


def build_gau_nc(s: int = S, debug: bool = False):
    nc = bacc.Bacc("TRN2", target_bir_lowering=False, debug=debug, num_devices=NCORES)
    io = {
        "hsT": nc.dram_tensor("hsT", [128, (D // 128) * s], BF16, kind="ExternalInput").ap(),
        "wq": nc.dram_tensor("wq", [128, (D // 128) * GD], BF16, kind="ExternalInput").ap(),
        "wk": nc.dram_tensor("wk", [128, (D // 128) * GD], BF16, kind="ExternalInput").ap(),
        "wv": nc.dram_tensor("wv", [128, (D // 128) * GD], BF16, kind="ExternalInput").ap(),
        "wo": nc.dram_tensor("wo", [GD, D], BF16, kind="ExternalInput").ap(),
        "bq": nc.dram_tensor("bq", [GD], F32, kind="ExternalInput").ap(),
        "bk": nc.dram_tensor("bk", [GD], F32, kind="ExternalInput").ap(),
        "bv": nc.dram_tensor("bv", [128, GD], F32, kind="ExternalInput").ap(),
        "outT": nc.dram_tensor("outT", [D, s], BF16, kind="ExternalOutput").ap(),
    }
    with tile.TileContext(nc) as tc:
        with ExitStack() as ctx:
            _build(ctx, tc, io, s)
    nc.compile()
    return nc


def make_in_maps(hidden_states, Wq, bq, Wk, bk, Wv, bv, Wo, gating_factor, gating_bias):
    """Shard full inputs into 8 per-core input maps (host-side prep)."""
    bf = ml_dtypes.bfloat16
    f32 = np.float32
    hs = np.asarray(hidden_states, f32)
    Wq, Wk, Wv, Wo = (np.asarray(a, f32) for a in (Wq, Wk, Wv, Wo))
    bq, bk, bv = (np.asarray(a, f32) for a in (bq, bk, bv))

    # hs^T pre-shuffled to [128, (chunk, d, 512)] so the device DMA is a
    # plain contiguous 2D block; weights to [128, (d, GD)] likewise
    def shuf_hs(a):  # a: [S, D]
        return np.ascontiguousarray(
            a.reshape(S // 512, 512, D // 128, 128).transpose(3, 0, 2, 1).reshape(128, -1)
        ).astype(bf)

    def shuf_w(w):  # w: [D, GD]
        return np.ascontiguousarray(
            w.reshape(D // 128, 128, GD).transpose(1, 0, 2).reshape(128, -1)
        ).astype(bf)

    hsT_b = [shuf_hs(hs[b]) for b in range(B)]
    in_maps = []
    for c in range(NCORES):
        b, g = divmod(c, NCORES // B)
        cols = slice(g * GD, (g + 1) * GD)
        in_maps.append(
            {
                "hsT": hsT_b[b],
                "wq": shuf_w(Wq[:, cols] * np.float32(LOG2E / 8.0)),
                "wk": shuf_w(Wk[:, cols]),
                "wv": shuf_w(Wv[:, cols]),
                "wo": np.ascontiguousarray(Wo[cols, :]).astype(bf),
                "bq": np.ascontiguousarray(bq[cols] * np.float32(LOG2E / 8.0)),
                "bk": np.ascontiguousarray(bk[cols]),
                "bv": np.ascontiguousarray(np.broadcast_to(bv[cols], (128, GD))),
            }
        )
    return in_maps


_NC_CACHE: dict = {}


def _get_nc(s: int = S):
    if s not in _NC_CACHE:
        _NC_CACHE[s] = build_gau_nc(s)
    return _NC_CACHE[s]


def run_gau(in_maps, **kwargs):
    nc = _get_nc(S)
    return run_bass_kernel_spmd(nc, in_maps, core_ids=list(range(NCORES)), **kwargs)


def assemble_output(results, bo, gating):
    """Sum per-batch head-group partials, apply gating, transpose, add bo."""
    bo = np.asarray(bo, np.float32)
    gpb = NCORES // B
    out = np.empty((B, S, D), np.float32)
    for b in range(B):
        acc = results[gpb * b]["outT"].astype(np.float32)
        for g in range(1, gpb):
            acc = acc + results[gpb * b + g]["outT"].astype(np.float32)
        out[b] = acc.T * gating[b][:, None] + bo[None, :]
    return out


def kernel(hidden_states, Wq, bq, Wk, bk, Wv, bv, Wo, bo, gating_factor, gating_bias):
    in_maps = make_in_maps(
        hidden_states, Wq, bq, Wk, bk, Wv, bv, Wo, gating_factor, gating_bias
    )
    hs = np.asarray(hidden_states, np.float32)
    gf = np.float32(np.asarray(gating_factor, np.float32)[0])
    gb = np.float32(np.asarray(gating_bias, np.float32)[0])
    gating = 1.0 / (1.0 + np.exp(-(gf * hs.mean(axis=-1) + gb)))  # [B, S]
    res = run_gau(in_maps)
    return assemble_output(res.results, bo, gating)


# revision 37
# speedup vs baseline: 1.0324x; 1.0103x over previous
"""GAU attention (gated attention unit) Trainium2 Bass kernel.

Reference computation (B=2, S=2048, D=1024, H=16, DH=64):
    q = (hs @ Wq + bq), k = (hs @ Wk + bk), v = (hs @ Wv + bv)   per-head [B,S,H,DH]
    scores = q k^T / sqrt(DH);  probs = softmax(scores, axis=k)
    gating = sigmoid(gf * mean_d(hs) + gb)          # [B, S] per (batch, query)
    ctx = (probs * gating) @ v;  out = ctx @ Wo + bo

Sharding: 8 cores = 2 batches x 4 head-groups (4 heads each).  Each core
computes out^T partial [D, S] for its (batch, head-group); host sums the 4
partials per batch, applies the per-(batch,query) gating scalar (it commutes
to the output), transposes, and adds bo.

Per-core dataflow (all matmuls bf16 with fp32 PSUM accumulation):
  - hs^T [D,S] staged bf16 (host transposes + casts).
  - Q^T,K^T [256,S]: lhsT=W tiles (stationary), rhs=hs^T.  Layout: pair p of
    heads stacked on partitions (head A dh on 0:64, head B on 64:128).
  - K-proj/V-proj/Q-proj are emitted just-in-time inside the first q-chunk's
    attention loop so the exp (ACT) stream starts ~10us into the kernel.
  - scores^T [k,q] per (pr, kt): two row-packed (tile_position (0,0)/(64,0))
    K=64 matmuls -> exp on ACT (scale=1/8) -> E^T bf16.  Both head-pair
    groups (pr=0,1) are interleaved per kt so ACT stays saturated.
  - softmax denom: E^T ktiles folded into 2 partial sums per pr -- even kt
    on DVE, odd kt on GPSIMD (parallel engines) -- then one col-packed
    ones-matmul pair broadcasts both heads' denominators into one [128,GQ]
    PSUM tile (accumulating even+odd partials), one reciprocal, one multiply.
  - AV: col-packed (tile_position (0,0)/(0,64)) matmuls, V stationary,
    E^T streaming -> ctx^T accumulated over ktiles in a single PSUM bank per
    pr (disjoint partition ranges; skip_group_check).
  - O-proj lhsT=Wo, rhs=ctx^T (bf16, already 1/denom-scaled) -> out^T.
"""

import sys

for _p in ("/opt/trn_rl_repo", "/root/.axon_site/_ro/trn_rl_repo"):
    if _p not in sys.path:
        sys.path.append(_p)

from contextlib import ExitStack

import ml_dtypes
import numpy as np

import concourse.bass as bass
import concourse.mybir as mybir
import concourse.tile as tile
from concourse import bacc
from concourse.bass_utils import run_bass_kernel_spmd

BF16 = mybir.dt.bfloat16
F32 = mybir.dt.float32
AF = mybir.ActivationFunctionType
OP = mybir.AluOpType

B, S, D, H = 2, 2048, 1024, 16
DH = 64
LN2 = float(np.log(2.0))
LOG2E = float(np.log2(np.e))
HPC = 4  # heads per core
GD = HPC * DH  # 256 (head-group width)
NCORES = 8
NDT = D // 128  # 8 contraction tiles over D


def _build(ctx: ExitStack, tc: "tile.TileContext", io: dict, s: int):
    nc = tc.nc
    GQ = min(512, s)
    NQC = s // GQ  # q chunks
    NKT = s // 128  # k tiles

    hsT, wq, wk, wv, wo = io["hsT"], io["wq"], io["wk"], io["wv"], io["wo"]
    bq, bk, bv, outT = io["bq"], io["bk"], io["bv"], io["outT"]

    consts = ctx.enter_context(tc.tile_pool(name="consts", bufs=1))
    sb = ctx.enter_context(tc.tile_pool(name="sb", bufs=1))
    etp = ctx.enter_context(tc.tile_pool(name="etp", bufs=8))
    ksp = ctx.enter_context(tc.tile_pool(name="ksp", bufs=2))
    outp = ctx.enter_context(tc.tile_pool(name="outp", bufs=8))
    # PSUM budget: 2x2 (scores, 2-bank tiles) + 2 (ctx, one bank per pr via
    # disjoint-partition accumulation groups) + 2 (vproj/denom/o-proj) = 8
    ps_mm = ctx.enter_context(tc.tile_pool(name="ps_mm", bufs=2, space="PSUM"))
    ps_ctx = ctx.enter_context(tc.tile_pool(name="ps_ctx", bufs=2, space="PSUM"))
    ps_o = ctx.enter_context(tc.tile_pool(name="ps_o", bufs=2, space="PSUM"))

    # ---- constants ----
    ones128 = consts.tile([128, 128], BF16, tag="ones128", name="ones128")
    nc.vector.memset(ones128[:], 1.0)

    bq_sb = consts.tile([128, 2], F32, tag="bq", name="bq")
    bk_sb = consts.tile([128, 2], F32, tag="bk", name="bk")
    # explicit zero bias for Exp, written by DVE so the wait merges with the
    # DVE wait the exps already carry
    zbias = consts.tile([128, 1], F32, tag="zbias", name="zbias")
    nc.vector.memset(zbias[:], 0.0)
    # dummy exp as the very first ACT instruction: pulls the ~2.7us
    # ACT_TABLE_LOAD into the DMA-wait window
    warm = consts.tile([1, 1], F32, tag="warm", name="warm")
    nc.scalar.activation(warm[:], zbias[0:1, 0:1], AF.Exp, bias=zbias[0:1, 0:1], scale=1.0)

    # bv arrives pre-broadcast [128, GD] from the host
    bv_bc = consts.tile([128, GD], F32, tag="bvbc", name="bvbc")

    # ---- weights + hs^T staged: host pre-shuffles so every DMA is a large
    # fully-contiguous 2D block (4KB+ per partition line).  Weight tensors
    # live as [128, (d, GD)]; hs^T as [128, (chunk, d, GQ)]. ----
    wk_all = consts.tile([128, NDT * GD], BF16, tag="wk", name="wk")
    wq_all = consts.tile([128, NDT * GD], BF16, tag="wq", name="wq")
    wv_all = consts.tile([128, NDT * GD], BF16, tag="wv", name="wv")
    CW = NDT * GQ  # 4096 columns per hs chunk block
    hsT_all = sb.tile([128, NDT * s], BF16, tag="hsT", name="hsT")
    wk_sb = [wk_all[:, d * GD : (d + 1) * GD] for d in range(NDT)]
    wq_sb = [wq_all[:, d * GD : (d + 1) * GD] for d in range(NDT)]
    wv_sb = [wv_all[:, d * GD : (d + 1) * GD] for d in range(NDT)]

    def hsq(d, qc):  # [128, GQ] tile of hs^T for (d-tile, q-chunk)
        off = qc * CW + d * GQ
        return hsT_all[:, off : off + GQ]

    def hsv(d, kt):  # [128, 128] tile of hs^T for (d-tile, k-tile)
        c, r = divmod(kt, 4)
        off = c * CW + d * GQ + r * 128
        return hsT_all[:, off : off + 128]

    # critical-path pieces (wk + hs chunk0 + wq) split in halves across the
    # two HW DGE rings; the scalar (ACT) ring carries ONLY critical pieces so
    # the exp stream isn't queued behind DMA issues
    HW_ = 4 * GD  # half of a weight block
    nc.sync.dma_start(wk_all[:, 0:HW_], wk[:, 0:HW_])
    nc.sync.dma_start(hsT_all[:, 0 : CW // 2], hsT[:, 0 : CW // 2])
    nc.sync.dma_start(wq_all[:, 0:HW_], wq[:, 0:HW_])
    nc.scalar.dma_start(wk_all[:, HW_ : 2 * HW_], wk[:, HW_ : 2 * HW_])
    nc.scalar.dma_start(hsT_all[:, CW // 2 : CW], hsT[:, CW // 2 : CW])
    nc.scalar.dma_start(wq_all[:, HW_ : 2 * HW_], wq[:, HW_ : 2 * HW_])
    nc.sync.dma_start(bq_sb[:], bq.rearrange("(m p) -> p m", p=128))
    nc.sync.dma_start(bk_sb[:], bk.rearrange("(m p) -> p m", p=128))
    nc.sync.dma_start(wv_all[:], wv[:, :])
    nc.sync.dma_start(bv_bc[:], bv[:, :])
    for c in range(1, NQC):
        nc.sync.dma_start(hsT_all[:, c * CW : (c + 1) * CW], hsT[:, c * CW : (c + 1) * CW])
    wo_sb = [consts.tile([128, D], BF16, tag=f"wo{p}", name=f"wo{p}") for p in range(2)]
    for p in range(2):
        nc.sync.dma_start(wo_sb[p][:], wo[p * 128 : (p + 1) * 128, :])

    qT_sb = [sb.tile([128, s], BF16, tag=f"qT{m}", name=f"qT{m}") for m in range(2)]
    kT_sb = [sb.tile([128, s], BF16, tag=f"kT{m}", name=f"kT{m}") for m in range(2)]
    v_sb = [sb.tile([128, GD], BF16, tag=f"v{st}", name=f"v{st}") for st in range(NKT)]

    def kproj(m, c):
        ms = slice(m * 128, (m + 1) * 128)
        cc = slice(c * GQ, (c + 1) * GQ)
        p = ps_o.tile([128, GQ], F32, tag="po", name=f"kp{m}{c}")
        for d in range(NDT):
            nc.tensor.matmul(
                p[:], lhsT=wk_sb[d][:, ms], rhs=hsq(d, c),
                start=(d == 0), stop=(d == NDT - 1),
            )
        nc.vector.tensor_scalar_add(kT_sb[m][:, cc], p[:], bk_sb[:, m : m + 1])

    def qproj(m, qc):
        ms = slice(m * 128, (m + 1) * 128)
        cc = slice(qc * GQ, (qc + 1) * GQ)
        p = ps_o.tile([128, GQ], F32, tag="po", name=f"qp{m}{qc}")
        for d in range(NDT):
            nc.tensor.matmul(
                p[:], lhsT=wq_sb[d][:, ms], rhs=hsq(d, qc),
                start=(d == 0), stop=(d == NDT - 1),
            )
        nc.vector.tensor_scalar_add(qT_sb[m][:, cc], p[:], bq_sb[:, m : m + 1])

    qchains = {}

    def qproj_half(m, qc, half):
        ms = slice(m * 128, (m + 1) * 128)
        if half == 0:
            qchains[(m, qc)] = ps_o.tile([128, GQ], F32, tag="po", name=f"qph{m}{qc}")
        p = qchains[(m, qc)]
        for d in range(half * 4, half * 4 + 4):
            nc.tensor.matmul(
                p[:], lhsT=wq_sb[d][:, ms], rhs=hsq(d, qc),
                start=(d == 0), stop=(d == NDT - 1),
            )
        if half == 1:
            cc = slice(qc * GQ, (qc + 1) * GQ)
            nc.vector.tensor_scalar_add(qT_sb[m][:, cc], p[:], bq_sb[:, m : m + 1])

    def vproj(kt):
        vp = ps_o.tile([128, GD], F32, tag="po", name=f"vp{kt}")
        for d in range(NDT):
            nc.tensor.matmul(
                vp[:], lhsT=hsv(d, kt), rhs=wv_sb[d][:],
                start=(d == 0), stop=(d == NDT - 1),
            )
        nc.vector.tensor_tensor(v_sb[kt][:], vp[:], bv_bc[:], op=OP.add)

    # ---- PE warm-up: ~3.5us of dependency-free matmuls so the HAM clock
    # gate opens before the real (DMA-gated) projections run ----
    warm_ps = ps_o.tile([128, 128], F32, tag="po", name="warm_ps")
    for i in range(24):
        nc.tensor.matmul(
            warm_ps[:], lhsT=ones128[:], rhs=ones128[:],
            start=(i == 0), stop=(i == 23),
        )

    # ---- prologue: just the pr0 projections; pr1's come as slot-0 filler
    # so the first exp fires after only two projection chains ----
    kproj(0, 0)
    qproj(0, 0)

    def oproj_unit(mt, ctx_sc, cs, copy_eng, dma_eng):
        ms = slice(mt * 128, (mt + 1) * 128)
        o_ps = ps_o.tile([128, GQ], F32, tag="po", name="po")
        for pr in range(2):
            nc.tensor.matmul(
                o_ps[:], lhsT=wo_sb[pr][:, ms], rhs=ctx_sc[pr][:],
                start=(pr == 0), stop=(pr == 1),
            )
        ost = outp.tile([128, GQ], BF16, tag="ost", name="ost")
        copy_eng(ost[:], o_ps[:])
        dma_eng.dma_start(outT[ms, cs], ost[:])

    # ---- per q-chunk attention, both head-pair groups interleaved per kt;
    # O-proj of chunk qc-1 trickles into qc's slots as PE filler ----
    oproj_pending: list = []
    for qc in range(NQC):
        cs = slice(qc * GQ, (qc + 1) * GQ)
        ctx_ps = [ps_ctx.tile([128, GQ], F32, tag="ctx", name=f"ctx{pr}") for pr in range(2)]
        # per (pr, kt-parity) partial exp-sums, folded on DVE
        ks = [[None, None], [None, None]]
        ets = [[None] * NKT, [None] * NKT]
        for kt in range(NKT + 1):
            sps = [None, None]
            for pr in range(2):
                if kt < NKT:
                    ks_ = slice(kt * 128, (kt + 1) * 128)
                    sp = ps_mm.tile([128, 2 * GQ], F32, tag="smm", name="smm")
                    sps[pr] = sp
                    nc.tensor.matmul(
                        sp[:, 0:GQ], lhsT=kT_sb[pr][0:64, ks_], rhs=qT_sb[pr][0:64, cs],
                        tile_position=(0, 0), start=True, stop=True,
                    )
                    nc.tensor.matmul(
                        sp[:, GQ : 2 * GQ], lhsT=kT_sb[pr][64:128, ks_], rhs=qT_sb[pr][64:128, cs],
                        tile_position=(64, 0), start=True, stop=True,
                    )
                if pr == 0 and kt < NKT:
                    # just-in-time projections keep PE fed while ACT drains exps
                    if qc == 0:
                        if kt == 0:
                            kproj(1, 0)
                            qproj(1, 0)
                            vproj(0)
                        if kt + 1 < NKT:
                            vproj(kt + 1)
                        if kt in (1, 5, 9):
                            kproj(0, kt // 4 + 1)
                        elif kt in (2, 6, 10):
                            kproj(1, kt // 4 + 1)
                        elif kt == 12:
                            qproj(0, 1)
                        elif kt == 13:
                            qproj(1, 1)
                    else:
                        if qc < NQC - 1:
                            if kt in (5, 6):
                                qproj_half(0, qc + 1, kt - 5)
                            elif kt in (7, 8):
                                qproj_half(1, qc + 1, kt - 7)
                        if oproj_pending and kt >= 1 and (kt < 5 or kt > 8):
                            oproj_pending.pop(0)()
                if kt > 0:
                    pv = kt - 1
                    et = ets[pr][pv]
                    nc.tensor.matmul(
                        ctx_ps[pr][0:64, :], lhsT=v_sb[pv][:, pr * 128 : pr * 128 + 64],
                        rhs=et[:, 0:GQ], tile_position=(0, 0),
                        start=(pv == 0), stop=(pv == NKT - 1),
                        skip_group_check=True,
                    )
                    nc.tensor.matmul(
                        ctx_ps[pr][64:128, :], lhsT=v_sb[pv][:, pr * 128 + 64 : pr * 128 + 128],
                        rhs=et[:, GQ : 2 * GQ], tile_position=(0, 64),
                        start=(pv == 0), stop=(pv == NKT - 1),
                        skip_group_check=True,
                    )
            for pr in range(2):
                if kt < NKT:
                    et = etp.tile([128, 2 * GQ], BF16, tag="et", name="et")
                    ets[pr][kt] = et
                    # scores arrive in log2 space (log2e/8 folded into Wq):
                    # exp(ln2*y) = 2^y on ACT
                    nc.scalar.activation(et[:], sps[pr][:], AF.Exp, bias=zbias[:, 0:1], scale=LN2)
                    par = kt % 2
                    if kt >= 2:
                        if kt < 4:
                            # first fold is out-of-place (no seed copy needed)
                            kst = ksp.tile([128, 2 * GQ], BF16, tag=f"ks{pr}{par}", name=f"ks{pr}{par}")
                            ks[pr][par] = kst
                            nc.vector.tensor_tensor(kst[:], ets[pr][par][:], et[:], op=OP.add)
                        else:
                            kst = ks[pr][par]
                            nc.vector.tensor_tensor(kst[:], kst[:], et[:], op=OP.add)

        # softmax denominators: col-packed ones-matmul pair broadcasts both
        # heads into one bank, accumulating even+odd partial sums
        def denom_ctx(pr):
            db = ps_o.tile([128, GQ], F32, tag="po", name=f"db{pr}")
            for par in range(2):
                for hh in range(2):
                    nc.tensor.matmul(
                        db[hh * 64 : (hh + 1) * 64, :],
                        lhsT=ones128[:, hh * 64 : (hh + 1) * 64],
                        rhs=ks[pr][par][:, hh * GQ : (hh + 1) * GQ],
                        tile_position=(0, hh * 64),
                        start=(par == 0), stop=(par == 1),
                        skip_group_check=True,
                    )
            r = ksp.tile([128, GQ], F32, tag=f"r{pr}", name=f"r{pr}")
            nc.vector.reciprocal_approx_fast(r[:], db[:])
            sc = sb.tile([128, GQ], BF16, tag=f"ctxs{pr}_{qc % 2}", name=f"ctxs{pr}_{qc % 2}")
            nc.vector.tensor_tensor(sc[:], ctx_ps[pr][:], r[:], op=OP.mult)
            return sc

        if qc < NQC - 1:
            # deferred into the next chunk's slots as PE filler
            ctx_sc = [denom_ctx(0), denom_ctx(1)]
            oproj_pending = [
                (lambda mt=mt, sc2=list(ctx_sc), c=cs: oproj_unit(
                    mt, sc2, c, nc.vector.tensor_copy, nc.sync))
                for mt in range(D // 128)
            ]
        else:
            # tail: pr0's half of the O-projection overlaps pr1's softmax
            # drain; pr1's half is added on DVE, outputs on both DMA rings
            sc0 = denom_ctx(0)
            osts = []
            for mt in range(D // 128):
                o_ps = ps_o.tile([128, GQ], F32, tag="po", name="po")
                nc.tensor.matmul(o_ps[:], lhsT=wo_sb[0][:, mt * 128 : (mt + 1) * 128],
                                 rhs=sc0[:], start=True, stop=True)
                ost = outp.tile([128, GQ], BF16, tag="ost", name="ost")
                copy_eng = nc.vector.tensor_copy if mt % 2 == 0 else nc.scalar.copy
                copy_eng(ost[:], o_ps[:])
                osts.append(ost)
            sc1 = denom_ctx(1)
            for mt in range(D // 128):
                ms = slice(mt * 128, (mt + 1) * 128)
                o_ps = ps_o.tile([128, GQ], F32, tag="po", name="po")
                nc.tensor.matmul(o_ps[:], lhsT=wo_sb[1][:, ms], rhs=sc1[:],
                                 start=True, stop=True)
                nc.vector.tensor_tensor(osts[mt][:], osts[mt][:], o_ps[:], op=OP.add)
                dma_eng = nc.sync if mt % 2 == 0 else nc.scalar
                dma_eng.dma_start(outT[ms, cs], osts[mt][:])


def build_gau_nc(s: int = S, debug: bool = False):
    nc = bacc.Bacc("TRN2", target_bir_lowering=False, debug=debug, num_devices=NCORES)
    io = {
        "hsT": nc.dram_tensor("hsT", [128, (D // 128) * s], BF16, kind="ExternalInput").ap(),
        "wq": nc.dram_tensor("wq", [128, (D // 128) * GD], BF16, kind="ExternalInput").ap(),
        "wk": nc.dram_tensor("wk", [128, (D // 128) * GD], BF16, kind="ExternalInput").ap(),
        "wv": nc.dram_tensor("wv", [128, (D // 128) * GD], BF16, kind="ExternalInput").ap(),
        "wo": nc.dram_tensor("wo", [GD, D], BF16, kind="ExternalInput").ap(),
        "bq": nc.dram_tensor("bq", [GD], F32, kind="ExternalInput").ap(),
        "bk": nc.dram_tensor("bk", [GD], F32, kind="ExternalInput").ap(),
        "bv": nc.dram_tensor("bv", [128, GD], F32, kind="ExternalInput").ap(),
        "outT": nc.dram_tensor("outT", [D, s], BF16, kind="ExternalOutput").ap(),
    }
    with tile.TileContext(nc) as tc:
        with ExitStack() as ctx:
            _build(ctx, tc, io, s)
    nc.compile()
    return nc


def make_in_maps(hidden_states, Wq, bq, Wk, bk, Wv, bv, Wo, gating_factor, gating_bias):
    """Shard full inputs into 8 per-core input maps (host-side prep)."""
    bf = ml_dtypes.bfloat16
    f32 = np.float32
    hs = np.asarray(hidden_states, f32)
    Wq, Wk, Wv, Wo = (np.asarray(a, f32) for a in (Wq, Wk, Wv, Wo))
    bq, bk, bv = (np.asarray(a, f32) for a in (bq, bk, bv))

    # hs^T pre-shuffled to [128, (chunk, d, 512)] so the device DMA is a
    # plain contiguous 2D block; weights to [128, (d, GD)] likewise
    def shuf_hs(a):  # a: [S, D]
        return np.ascontiguousarray(
            a.reshape(S // 512, 512, D // 128, 128).transpose(3, 0, 2, 1).reshape(128, -1)
        ).astype(bf)

    def shuf_w(w):  # w: [D, GD]
        return np.ascontiguousarray(
            w.reshape(D // 128, 128, GD).transpose(1, 0, 2).reshape(128, -1)
        ).astype(bf)

    hsT_b = [shuf_hs(hs[b]) for b in range(B)]
    in_maps = []
    for c in range(NCORES):
        b, g = divmod(c, NCORES // B)
        cols = slice(g * GD, (g + 1) * GD)
        in_maps.append(
            {
                "hsT": hsT_b[b],
                "wq": shuf_w(Wq[:, cols] * np.float32(LOG2E / 8.0)),
                "wk": shuf_w(Wk[:, cols]),
                "wv": shuf_w(Wv[:, cols]),
                "wo": np.ascontiguousarray(Wo[cols, :]).astype(bf),
                "bq": np.ascontiguousarray(bq[cols] * np.float32(LOG2E / 8.0)),
                "bk": np.ascontiguousarray(bk[cols]),
                "bv": np.ascontiguousarray(np.broadcast_to(bv[cols], (128, GD))),
            }
        )
    return in_maps


_NC_CACHE: dict = {}


def _get_nc(s: int = S):
    if s not in _NC_CACHE:
        _NC_CACHE[s] = build_gau_nc(s)
    return _NC_CACHE[s]


def run_gau(in_maps, **kwargs):
    nc = _get_nc(S)
    return run_bass_kernel_spmd(nc, in_maps, core_ids=list(range(NCORES)), **kwargs)


def assemble_output(results, bo, gating):
    """Sum per-batch head-group partials, apply gating, transpose, add bo."""
    bo = np.asarray(bo, np.float32)
    gpb = NCORES // B
    out = np.empty((B, S, D), np.float32)
    for b in range(B):
        acc = results[gpb * b]["outT"].astype(np.float32)
        for g in range(1, gpb):
            acc = acc + results[gpb * b + g]["outT"].astype(np.float32)
        out[b] = acc.T * gating[b][:, None] + bo[None, :]
    return out


def kernel(hidden_states, Wq, bq, Wk, bk, Wv, bv, Wo, bo, gating_factor, gating_bias):
    in_maps = make_in_maps(
        hidden_states, Wq, bq, Wk, bk, Wv, bv, Wo, gating_factor, gating_bias
    )
    hs = np.asarray(hidden_states, np.float32)
    gf = np.float32(np.asarray(gating_factor, np.float32)[0])
    gb = np.float32(np.asarray(gating_bias, np.float32)[0])
    gating = 1.0 / (1.0 + np.exp(-(gf * hs.mean(axis=-1) + gb)))  # [B, S]
    res = run_gau(in_maps)
    return assemble_output(res.results, bo, gating)


# revision 38
# speedup vs baseline: 1.0352x; 1.0027x over previous
"""GAU attention (gated attention unit) Trainium2 Bass kernel.

Reference computation (B=2, S=2048, D=1024, H=16, DH=64):
    q = (hs @ Wq + bq), k = (hs @ Wk + bk), v = (hs @ Wv + bv)   per-head [B,S,H,DH]
    scores = q k^T / sqrt(DH);  probs = softmax(scores, axis=k)
    gating = sigmoid(gf * mean_d(hs) + gb)          # [B, S] per (batch, query)
    ctx = (probs * gating) @ v;  out = ctx @ Wo + bo

Sharding: 8 cores = 2 batches x 4 head-groups (4 heads each).  Each core
computes out^T partial [D, S] for its (batch, head-group); host sums the 4
partials per batch, applies the per-(batch,query) gating scalar (it commutes
to the output), transposes, and adds bo.

Per-core dataflow (all matmuls bf16 with fp32 PSUM accumulation):
  - hs^T [D,S] staged bf16 (host transposes + casts).
  - Q^T,K^T [256,S]: lhsT=W tiles (stationary), rhs=hs^T.  Layout: pair p of
    heads stacked on partitions (head A dh on 0:64, head B on 64:128).
  - K-proj/V-proj/Q-proj are emitted just-in-time inside the first q-chunk's
    attention loop so the exp (ACT) stream starts ~10us into the kernel.
  - scores^T [k,q] per (pr, kt): two row-packed (tile_position (0,0)/(64,0))
    K=64 matmuls -> exp on ACT (scale=1/8) -> E^T bf16.  Both head-pair
    groups (pr=0,1) are interleaved per kt so ACT stays saturated.
  - softmax denom: E^T ktiles folded into 2 partial sums per pr -- even kt
    on DVE, odd kt on GPSIMD (parallel engines) -- then one col-packed
    ones-matmul pair broadcasts both heads' denominators into one [128,GQ]
    PSUM tile (accumulating even+odd partials), one reciprocal, one multiply.
  - AV: col-packed (tile_position (0,0)/(0,64)) matmuls, V stationary,
    E^T streaming -> ctx^T accumulated over ktiles in a single PSUM bank per
    pr (disjoint partition ranges; skip_group_check).
  - O-proj lhsT=Wo, rhs=ctx^T (bf16, already 1/denom-scaled) -> out^T.
"""

import sys

for _p in ("/opt/trn_rl_repo", "/root/.axon_site/_ro/trn_rl_repo"):
    if _p not in sys.path:
        sys.path.append(_p)

from contextlib import ExitStack

import ml_dtypes
import numpy as np

import concourse.bass as bass
import concourse.mybir as mybir
import concourse.tile as tile
from concourse import bacc
from concourse.bass_utils import run_bass_kernel_spmd

BF16 = mybir.dt.bfloat16
F32 = mybir.dt.float32
AF = mybir.ActivationFunctionType
OP = mybir.AluOpType

B, S, D, H = 2, 2048, 1024, 16
DH = 64
LN2 = float(np.log(2.0))
LOG2E = float(np.log2(np.e))
HPC = 4  # heads per core
GD = HPC * DH  # 256 (head-group width)
NCORES = 8
NDT = D // 128  # 8 contraction tiles over D


def _build(ctx: ExitStack, tc: "tile.TileContext", io: dict, s: int):
    nc = tc.nc
    GQ = min(512, s)
    NQC = s // GQ  # q chunks
    NKT = s // 128  # k tiles

    hsT, wq, wk, wv, wo = io["hsT"], io["wq"], io["wk"], io["wv"], io["wo"]
    bq, bk, bv, outT = io["bq"], io["bk"], io["bv"], io["outT"]

    consts = ctx.enter_context(tc.tile_pool(name="consts", bufs=1))
    sb = ctx.enter_context(tc.tile_pool(name="sb", bufs=1))
    etp = ctx.enter_context(tc.tile_pool(name="etp", bufs=8))
    ksp = ctx.enter_context(tc.tile_pool(name="ksp", bufs=2))
    outp = ctx.enter_context(tc.tile_pool(name="outp", bufs=8))
    # PSUM budget: 2x2 (scores, 2-bank tiles) + 2 (ctx, one bank per pr via
    # disjoint-partition accumulation groups) + 2 (vproj/denom/o-proj) = 8
    ps_mm = ctx.enter_context(tc.tile_pool(name="ps_mm", bufs=2, space="PSUM"))
    ps_ctx = ctx.enter_context(tc.tile_pool(name="ps_ctx", bufs=2, space="PSUM"))
    ps_o = ctx.enter_context(tc.tile_pool(name="ps_o", bufs=2, space="PSUM"))

    # ---- constants ----
    ones128 = consts.tile([128, 128], BF16, tag="ones128", name="ones128")
    nc.vector.memset(ones128[:], 1.0)

    bq_sb = consts.tile([128, 2], F32, tag="bq", name="bq")
    bk_sb = consts.tile([128, 2], F32, tag="bk", name="bk")
    # explicit zero bias for Exp, written by DVE so the wait merges with the
    # DVE wait the exps already carry
    zbias = consts.tile([128, 1], F32, tag="zbias", name="zbias")
    nc.vector.memset(zbias[:], 0.0)
    # dummy exp as the very first ACT instruction: pulls the ~2.7us
    # ACT_TABLE_LOAD into the DMA-wait window
    warm = consts.tile([1, 1], F32, tag="warm", name="warm")
    nc.scalar.activation(warm[:], zbias[0:1, 0:1], AF.Exp, bias=zbias[0:1, 0:1], scale=1.0)

    # bv arrives pre-broadcast [128, GD] from the host
    bv_bc = consts.tile([128, GD], F32, tag="bvbc", name="bvbc")

    # ---- weights + hs^T staged: host pre-shuffles so every DMA is a large
    # fully-contiguous 2D block (4KB+ per partition line).  Weight tensors
    # live as [128, (d, GD)]; hs^T as [128, (chunk, d, GQ)]. ----
    wk_all = consts.tile([128, NDT * GD], BF16, tag="wk", name="wk")
    wq_all = consts.tile([128, NDT * GD], BF16, tag="wq", name="wq")
    wv_all = consts.tile([128, NDT * GD], BF16, tag="wv", name="wv")
    CW = NDT * GQ  # 4096 columns per hs chunk block
    hsT_all = sb.tile([128, NDT * s], BF16, tag="hsT", name="hsT")
    wk_sb = [wk_all[:, d * GD : (d + 1) * GD] for d in range(NDT)]
    wq_sb = [wq_all[:, d * GD : (d + 1) * GD] for d in range(NDT)]
    wv_sb = [wv_all[:, d * GD : (d + 1) * GD] for d in range(NDT)]

    def hsq(d, qc):  # [128, GQ] tile of hs^T for (d-tile, q-chunk)
        off = qc * CW + d * GQ
        return hsT_all[:, off : off + GQ]

    def hsv(d, kt):  # [128, 128] tile of hs^T for (d-tile, k-tile)
        c, r = divmod(kt, 4)
        off = c * CW + d * GQ + r * 128
        return hsT_all[:, off : off + 128]

    # critical-path pieces (wk + hs chunk0 + wq) split in halves across the
    # two HW DGE rings; the scalar (ACT) ring carries ONLY critical pieces so
    # the exp stream isn't queued behind DMA issues
    QW_ = 2 * GD  # quarter of a weight block (2 d-tiles)
    QH_ = CW // 4  # quarter of an hs chunk (2 d-tiles)
    for q4 in range(2):  # sync ring: d0-1,d2-3 pieces; scalar: d4-5,d6-7
        nc.sync.dma_start(wk_all[:, q4 * QW_ : (q4 + 1) * QW_], wk[:, q4 * QW_ : (q4 + 1) * QW_])
        nc.sync.dma_start(hsT_all[:, q4 * QH_ : (q4 + 1) * QH_], hsT[:, q4 * QH_ : (q4 + 1) * QH_])
        nc.scalar.dma_start(wk_all[:, (q4 + 2) * QW_ : (q4 + 3) * QW_], wk[:, (q4 + 2) * QW_ : (q4 + 3) * QW_])
        nc.scalar.dma_start(hsT_all[:, (q4 + 2) * QH_ : (q4 + 3) * QH_], hsT[:, (q4 + 2) * QH_ : (q4 + 3) * QH_])
    HW_ = 4 * GD
    nc.sync.dma_start(wq_all[:, 0:HW_], wq[:, 0:HW_])
    nc.scalar.dma_start(wq_all[:, HW_ : 2 * HW_], wq[:, HW_ : 2 * HW_])
    nc.sync.dma_start(bq_sb[:], bq.rearrange("(m p) -> p m", p=128))
    nc.sync.dma_start(bk_sb[:], bk.rearrange("(m p) -> p m", p=128))
    nc.sync.dma_start(wv_all[:], wv[:, :])
    nc.sync.dma_start(bv_bc[:], bv[:, :])
    for c in range(1, NQC):
        nc.sync.dma_start(hsT_all[:, c * CW : (c + 1) * CW], hsT[:, c * CW : (c + 1) * CW])
    wo_sb = [consts.tile([128, D], BF16, tag=f"wo{p}", name=f"wo{p}") for p in range(2)]
    for p in range(2):
        nc.sync.dma_start(wo_sb[p][:], wo[p * 128 : (p + 1) * 128, :])

    qT_sb = [sb.tile([128, s], BF16, tag=f"qT{m}", name=f"qT{m}") for m in range(2)]
    kT_sb = [sb.tile([128, s], BF16, tag=f"kT{m}", name=f"kT{m}") for m in range(2)]
    v_sb = [sb.tile([128, GD], BF16, tag=f"v{st}", name=f"v{st}") for st in range(NKT)]

    def kproj(m, c):
        ms = slice(m * 128, (m + 1) * 128)
        cc = slice(c * GQ, (c + 1) * GQ)
        p = ps_o.tile([128, GQ], F32, tag="po", name=f"kp{m}{c}")
        for d in range(NDT):
            nc.tensor.matmul(
                p[:], lhsT=wk_sb[d][:, ms], rhs=hsq(d, c),
                start=(d == 0), stop=(d == NDT - 1),
            )
        nc.vector.tensor_scalar_add(kT_sb[m][:, cc], p[:], bk_sb[:, m : m + 1])

    def qproj(m, qc):
        ms = slice(m * 128, (m + 1) * 128)
        cc = slice(qc * GQ, (qc + 1) * GQ)
        p = ps_o.tile([128, GQ], F32, tag="po", name=f"qp{m}{qc}")
        for d in range(NDT):
            nc.tensor.matmul(
                p[:], lhsT=wq_sb[d][:, ms], rhs=hsq(d, qc),
                start=(d == 0), stop=(d == NDT - 1),
            )
        nc.vector.tensor_scalar_add(qT_sb[m][:, cc], p[:], bq_sb[:, m : m + 1])

    qchains = {}

    def qproj_half(m, qc, half):
        ms = slice(m * 128, (m + 1) * 128)
        if half == 0:
            qchains[(m, qc)] = ps_o.tile([128, GQ], F32, tag="po", name=f"qph{m}{qc}")
        p = qchains[(m, qc)]
        for d in range(half * 4, half * 4 + 4):
            nc.tensor.matmul(
                p[:], lhsT=wq_sb[d][:, ms], rhs=hsq(d, qc),
                start=(d == 0), stop=(d == NDT - 1),
            )
        if half == 1:
            cc = slice(qc * GQ, (qc + 1) * GQ)
            nc.vector.tensor_scalar_add(qT_sb[m][:, cc], p[:], bq_sb[:, m : m + 1])

    def vproj(kt):
        vp = ps_o.tile([128, GD], F32, tag="po", name=f"vp{kt}")
        for d in range(NDT):
            nc.tensor.matmul(
                vp[:], lhsT=hsv(d, kt), rhs=wv_sb[d][:],
                start=(d == 0), stop=(d == NDT - 1),
            )
        nc.vector.tensor_tensor(v_sb[kt][:], vp[:], bv_bc[:], op=OP.add)

    # ---- PE warm-up: ~3.5us of dependency-free matmuls so the HAM clock
    # gate opens before the real (DMA-gated) projections run ----
    warm_ps = ps_o.tile([128, 128], F32, tag="po", name="warm_ps")
    for i in range(24):
        nc.tensor.matmul(
            warm_ps[:], lhsT=ones128[:], rhs=ones128[:],
            start=(i == 0), stop=(i == 23),
        )

    # ---- prologue: just the pr0 projections; pr1's come as slot-0 filler
    # so the first exp fires after only two projection chains ----
    kproj(0, 0)
    qproj(0, 0)

    def oproj_unit(mt, ctx_sc, cs, copy_eng, dma_eng):
        ms = slice(mt * 128, (mt + 1) * 128)
        o_ps = ps_o.tile([128, GQ], F32, tag="po", name="po")
        for pr in range(2):
            nc.tensor.matmul(
                o_ps[:], lhsT=wo_sb[pr][:, ms], rhs=ctx_sc[pr][:],
                start=(pr == 0), stop=(pr == 1),
            )
        ost = outp.tile([128, GQ], BF16, tag="ost", name="ost")
        copy_eng(ost[:], o_ps[:])
        dma_eng.dma_start(outT[ms, cs], ost[:])

    # ---- per q-chunk attention, both head-pair groups interleaved per kt;
    # O-proj of chunk qc-1 trickles into qc's slots as PE filler ----
    oproj_pending: list = []
    for qc in range(NQC):
        cs = slice(qc * GQ, (qc + 1) * GQ)
        ctx_ps = [ps_ctx.tile([128, GQ], F32, tag="ctx", name=f"ctx{pr}") for pr in range(2)]
        # per (pr, kt-parity) partial exp-sums, folded on DVE
        ks = [[None, None], [None, None]]
        ets = [[None] * NKT, [None] * NKT]
        for kt in range(NKT + 1):
            sps = [None, None]
            for pr in range(2):
                if kt < NKT:
                    ks_ = slice(kt * 128, (kt + 1) * 128)
                    sp = ps_mm.tile([128, 2 * GQ], F32, tag="smm", name="smm")
                    sps[pr] = sp
                    nc.tensor.matmul(
                        sp[:, 0:GQ], lhsT=kT_sb[pr][0:64, ks_], rhs=qT_sb[pr][0:64, cs],
                        tile_position=(0, 0), start=True, stop=True,
                    )
                    nc.tensor.matmul(
                        sp[:, GQ : 2 * GQ], lhsT=kT_sb[pr][64:128, ks_], rhs=qT_sb[pr][64:128, cs],
                        tile_position=(64, 0), start=True, stop=True,
                    )
                if pr == 0 and kt < NKT:
                    # just-in-time projections keep PE fed while ACT drains exps
                    if qc == 0:
                        if kt == 0:
                            kproj(1, 0)
                            qproj(1, 0)
                            vproj(0)
                        if kt + 1 < NKT:
                            vproj(kt + 1)
                        if kt in (1, 5, 9):
                            kproj(0, kt // 4 + 1)
                        elif kt in (2, 6, 10):
                            kproj(1, kt // 4 + 1)
                        elif kt == 12:
                            qproj(0, 1)
                        elif kt == 13:
                            qproj(1, 1)
                    else:
                        if qc < NQC - 1:
                            if kt in (5, 6):
                                qproj_half(0, qc + 1, kt - 5)
                            elif kt in (7, 8):
                                qproj_half(1, qc + 1, kt - 7)
                        if oproj_pending and kt >= 1 and (kt < 5 or kt > 8):
                            oproj_pending.pop(0)()
                if kt > 0:
                    pv = kt - 1
                    et = ets[pr][pv]
                    nc.tensor.matmul(
                        ctx_ps[pr][0:64, :], lhsT=v_sb[pv][:, pr * 128 : pr * 128 + 64],
                        rhs=et[:, 0:GQ], tile_position=(0, 0),
                        start=(pv == 0), stop=(pv == NKT - 1),
                        skip_group_check=True,
                    )
                    nc.tensor.matmul(
                        ctx_ps[pr][64:128, :], lhsT=v_sb[pv][:, pr * 128 + 64 : pr * 128 + 128],
                        rhs=et[:, GQ : 2 * GQ], tile_position=(0, 64),
                        start=(pv == 0), stop=(pv == NKT - 1),
                        skip_group_check=True,
                    )
            for pr in range(2):
                if kt < NKT:
                    et = etp.tile([128, 2 * GQ], BF16, tag="et", name="et")
                    ets[pr][kt] = et
                    # scores arrive in log2 space (log2e/8 folded into Wq):
                    # exp(ln2*y) = 2^y on ACT
                    nc.scalar.activation(et[:], sps[pr][:], AF.Exp, bias=zbias[:, 0:1], scale=LN2)
                    par = kt % 2
                    if kt >= 2:
                        if kt < 4:
                            # first fold is out-of-place (no seed copy needed)
                            kst = ksp.tile([128, 2 * GQ], BF16, tag=f"ks{pr}{par}", name=f"ks{pr}{par}")
                            ks[pr][par] = kst
                            nc.vector.tensor_tensor(kst[:], ets[pr][par][:], et[:], op=OP.add)
                        else:
                            kst = ks[pr][par]
                            nc.vector.tensor_tensor(kst[:], kst[:], et[:], op=OP.add)

        # softmax denominators: col-packed ones-matmul pair broadcasts both
        # heads into one bank, accumulating even+odd partial sums
        def denom_ctx(pr):
            db = ps_o.tile([128, GQ], F32, tag="po", name=f"db{pr}")
            for par in range(2):
                for hh in range(2):
                    nc.tensor.matmul(
                        db[hh * 64 : (hh + 1) * 64, :],
                        lhsT=ones128[:, hh * 64 : (hh + 1) * 64],
                        rhs=ks[pr][par][:, hh * GQ : (hh + 1) * GQ],
                        tile_position=(0, hh * 64),
                        start=(par == 0), stop=(par == 1),
                        skip_group_check=True,
                    )
            r = ksp.tile([128, GQ], F32, tag=f"r{pr}", name=f"r{pr}")
            nc.vector.reciprocal_approx_fast(r[:], db[:])
            sc = sb.tile([128, GQ], BF16, tag=f"ctxs{pr}_{qc % 2}", name=f"ctxs{pr}_{qc % 2}")
            nc.vector.tensor_tensor(sc[:], ctx_ps[pr][:], r[:], op=OP.mult)
            return sc

        if qc < NQC - 1:
            # deferred into the next chunk's slots as PE filler
            ctx_sc = [denom_ctx(0), denom_ctx(1)]
            oproj_pending = [
                (lambda mt=mt, sc2=list(ctx_sc), c=cs: oproj_unit(
                    mt, sc2, c, nc.vector.tensor_copy, nc.sync))
                for mt in range(D // 128)
            ]
        else:
            # tail: pr0's half of the O-projection overlaps pr1's softmax
            # drain; pr1's half is added on DVE, outputs on both DMA rings
            sc0 = denom_ctx(0)
            osts = []
            for mt in range(D // 128):
                o_ps = ps_o.tile([128, GQ], F32, tag="po", name="po")
                nc.tensor.matmul(o_ps[:], lhsT=wo_sb[0][:, mt * 128 : (mt + 1) * 128],
                                 rhs=sc0[:], start=True, stop=True)
                ost = outp.tile([128, GQ], BF16, tag="ost", name="ost")
                copy_eng = nc.vector.tensor_copy if mt % 2 == 0 else nc.scalar.copy
                copy_eng(ost[:], o_ps[:])
                osts.append(ost)
            sc1 = denom_ctx(1)
            for mt in range(D // 128):
                ms = slice(mt * 128, (mt + 1) * 128)
                o_ps = ps_o.tile([128, GQ], F32, tag="po", name="po")
                nc.tensor.matmul(o_ps[:], lhsT=wo_sb[1][:, ms], rhs=sc1[:],
                                 start=True, stop=True)
                nc.vector.tensor_tensor(osts[mt][:], osts[mt][:], o_ps[:], op=OP.add)
                dma_eng = nc.sync if mt % 2 == 0 else nc.scalar
                dma_eng.dma_start(outT[ms, cs], osts[mt][:])


def build_gau_nc(s: int = S, debug: bool = False):
    nc = bacc.Bacc("TRN2", target_bir_lowering=False, debug=debug, num_devices=NCORES)
    io = {
        "hsT": nc.dram_tensor("hsT", [128, (D // 128) * s], BF16, kind="ExternalInput").ap(),
        "wq": nc.dram_tensor("wq", [128, (D // 128) * GD], BF16, kind="ExternalInput").ap(),
        "wk": nc.dram_tensor("wk", [128, (D // 128) * GD], BF16, kind="ExternalInput").ap(),
        "wv": nc.dram_tensor("wv", [128, (D // 128) * GD], BF16, kind="ExternalInput").ap(),
        "wo": nc.dram_tensor("wo", [GD, D], BF16, kind="ExternalInput").ap(),
        "bq": nc.dram_tensor("bq", [GD], F32, kind="ExternalInput").ap(),
        "bk": nc.dram_tensor("bk", [GD], F32, kind="ExternalInput").ap(),
        "bv": nc.dram_tensor("bv", [128, GD], F32, kind="ExternalInput").ap(),
        "outT": nc.dram_tensor("outT", [D, s], BF16, kind="ExternalOutput").ap(),
    }
    with tile.TileContext(nc) as tc:
        with ExitStack() as ctx:
            _build(ctx, tc, io, s)
    nc.compile()
    return nc


def make_in_maps(hidden_states, Wq, bq, Wk, bk, Wv, bv, Wo, gating_factor, gating_bias):
    """Shard full inputs into 8 per-core input maps (host-side prep)."""
    bf = ml_dtypes.bfloat16
    f32 = np.float32
    hs = np.asarray(hidden_states, f32)
    Wq, Wk, Wv, Wo = (np.asarray(a, f32) for a in (Wq, Wk, Wv, Wo))
    bq, bk, bv = (np.asarray(a, f32) for a in (bq, bk, bv))

    # hs^T pre-shuffled to [128, (chunk, d, 512)] so the device DMA is a
    # plain contiguous 2D block; weights to [128, (d, GD)] likewise
    def shuf_hs(a):  # a: [S, D]
        return np.ascontiguousarray(
            a.reshape(S // 512, 512, D // 128, 128).transpose(3, 0, 2, 1).reshape(128, -1)
        ).astype(bf)

    def shuf_w(w):  # w: [D, GD]
        return np.ascontiguousarray(
            w.reshape(D // 128, 128, GD).transpose(1, 0, 2).reshape(128, -1)
        ).astype(bf)

    hsT_b = [shuf_hs(hs[b]) for b in range(B)]
    in_maps = []
    for c in range(NCORES):
        b, g = divmod(c, NCORES // B)
        cols = slice(g * GD, (g + 1) * GD)
        in_maps.append(
            {
                "hsT": hsT_b[b],
                "wq": shuf_w(Wq[:, cols] * np.float32(LOG2E / 8.0)),
                "wk": shuf_w(Wk[:, cols]),
                "wv": shuf_w(Wv[:, cols]),
                "wo": np.ascontiguousarray(Wo[cols, :]).astype(bf),
                "bq": np.ascontiguousarray(bq[cols] * np.float32(LOG2E / 8.0)),
                "bk": np.ascontiguousarray(bk[cols]),
                "bv": np.ascontiguousarray(np.broadcast_to(bv[cols], (128, GD))),
            }
        )
    return in_maps


_NC_CACHE: dict = {}


def _get_nc(s: int = S):
    if s not in _NC_CACHE:
        _NC_CACHE[s] = build_gau_nc(s)
    return _NC_CACHE[s]


def run_gau(in_maps, **kwargs):
    nc = _get_nc(S)
    return run_bass_kernel_spmd(nc, in_maps, core_ids=list(range(NCORES)), **kwargs)


def assemble_output(results, bo, gating):
    """Sum per-batch head-group partials, apply gating, transpose, add bo."""
    bo = np.asarray(bo, np.float32)
    gpb = NCORES // B
    out = np.empty((B, S, D), np.float32)
    for b in range(B):
        acc = results[gpb * b]["outT"].astype(np.float32)
        for g in range(1, gpb):
            acc = acc + results[gpb * b + g]["outT"].astype(np.float32)
        out[b] = acc.T * gating[b][:, None] + bo[None, :]
    return out


def kernel(hidden_states, Wq, bq, Wk, bk, Wv, bv, Wo, bo, gating_factor, gating_bias):
    in_maps = make_in_maps(
        hidden_states, Wq, bq, Wk, bk, Wv, bv, Wo, gating_factor, gating_bias
    )
    hs = np.asarray(hidden_states, np.float32)
    gf = np.float32(np.asarray(gating_factor, np.float32)[0])
    gb = np.float32(np.asarray(gating_bias, np.float32)[0])
    gating = 1.0 / (1.0 + np.exp(-(gf * hs.mean(axis=-1) + gb)))  # [B, S]
    res = run_gau(in_maps)
    return assemble_output(res.results, bo, gating)
